# revision 7
# baseline (speedup 1.0000x reference)
"""MoE (MiMo-V2) kernel for 8x Trainium2 NeuronCores.

Strategy (expert-parallel, per the sharding hint):
  - Host: grouped-topk routing (exact replica of the reference gate, run in
    fp32 on jax-cpu), then tokens are gathered per expert into fixed-capacity
    segments. Each of the 8 cores owns 8 experts.
  - Device (Bass/Tile, one SPMD program): for each local expert, stream its
    gathered tokens through gate/up matmuls (bf16 operands, fp32 PSUM
    accumulate), silu*mul on ACT/DVE, down matmul back to token-major
    layout, scale rows by the combine weights, write gathered rows out.
  - Host: scatter-add the gathered per-expert rows into the [T, H] output.
"""

import numpy as np
import ml_dtypes

T, H, E, I, K, G, KG = 16384, 1024, 64, 768, 8, 8, 4
P = 128
NCORES = 8
EPC = E // NCORES  # experts per core
HC = H // P  # 8 contraction chunks for gate/up
IC = I // P  # 6 contraction chunks for down
I2 = 2 * I  # fused gate+up output width

BF16 = ml_dtypes.bfloat16

_program_cache = {}
last_results = None  # BassKernelResults of the most recent launch (for test.py)


def _routing(hidden, gate_w, bias):
    """Exact replica of reference._grouped_topk on jax-cpu (fp32)."""
    import jax
    import jax.numpy as jnp

    cpu = jax.devices("cpu")[0]
    with jax.default_device(cpu):
        hidden = jnp.asarray(np.asarray(hidden), jnp.float32)
        gate_w = jnp.asarray(np.asarray(gate_w), jnp.float32)
        bias = jnp.asarray(np.asarray(bias), jnp.float32)
        logits = hidden @ gate_w.T
        scores = jax.nn.sigmoid(logits)
        s_choice = scores + bias[None, :]
        t, e = scores.shape
        grouped = s_choice.reshape(t, G, e // G)
        top2, _ = jax.lax.top_k(grouped, 2)
        group_scores = top2.sum(-1)
        _, gidx = jax.lax.top_k(group_scores, KG)
        gmask = jnp.zeros((t, G), jnp.float32).at[jnp.arange(t)[:, None], gidx].set(1.0)
        emask = jnp.repeat(gmask, e // G, axis=1)
        masked = jnp.where(emask > 0, s_choice, -jnp.inf)
        _, topk_idx = jax.lax.top_k(masked, K)
        topk_w = jnp.take_along_axis(scores, topk_idx, axis=1)
        topk_w = topk_w / (topk_w.sum(-1, keepdims=True) + 1e-20)
        return np.asarray(topk_idx), np.asarray(topk_w, np.float32)


def _build_program(slot_blocks):
    """One SPMD Bass program. slot_blocks[j] is the token-block decomposition
    of local-expert slot j; slots have (generally different) fixed capacities
    shared by all cores."""
    import concourse.mybir as mybir
    from concourse import bacc
    from concourse.tile import TileContext

    caps = [sum(b) for b in slot_blocks]
    seg_off = np.zeros(EPC + 1, np.int64)
    np.cumsum(caps, out=seg_off[1:])
    NC = int(seg_off[-1])
    bf = mybir.dt.bfloat16
    f32 = mybir.dt.float32
    Silu = mybir.ActivationFunctionType.Silu
    mult = mybir.AluOpType.mult

    nc = bacc.Bacc("TRN2", target_bir_lowering=False, debug=False, num_devices=NCORES)
    xgt = nc.dram_tensor("xgt", [H, NC], bf, kind="ExternalInput").ap()
    wgu = nc.dram_tensor("wgu", [EPC, H, I2], bf, kind="ExternalInput").ap()
    wd = nc.dram_tensor("wd", [EPC, I, H], bf, kind="ExternalInput").ap()
    cv = nc.dram_tensor("cv", [NC, 1], f32, kind="ExternalInput").ap()
    g = nc.dram_tensor("g", [NC, H], f32, kind="ExternalOutput").ap()

    with TileContext(nc) as tc:
        with (
            tc.tile_pool(name="wpool", bufs=2) as wpool,
            tc.tile_pool(name="xpool", bufs=2) as xpool,
            tc.tile_pool(name="apool", bufs=2) as apool,
            tc.tile_pool(name="spool", bufs=2) as spool,
            tc.tile_pool(name="opool", bufs=4) as opool,
            tc.tile_pool(name="cpool", bufs=4) as cpool,
            tc.tile_pool(name="psg", bufs=1, space="PSUM") as psg,
            tc.tile_pool(name="psu", bufs=1, space="PSUM") as psu,
            tc.tile_pool(name="pso", bufs=2, space="PSUM") as pso,
        ):
            xgt_r = xgt.rearrange("(c p) t -> p c t", p=P)  # [128, HC, NC]
            for ei in range(EPC):
                wgu_sb = wpool.tile([P, HC, I2], bf, tag="wgu")
                nc.sync.dma_start(
                    out=wgu_sb[:], in_=wgu[ei].rearrange("(c p) i -> p c i", p=P)
                )
                wd_sb = wpool.tile([P, IC, H], bf, tag="wd")
                nc.sync.dma_start(
                    out=wd_sb[:], in_=wd[ei].rearrange("(c p) h -> p c h", p=P)
                )
                off = 0
                for bn in slot_blocks[ei]:
                    s = int(seg_off[ei]) + off
                    # token sub-blocks of <=512 within this block; consecutive
                    # matmuls share one stationary (LDWEIGHTS) load across them
                    sbs = [
                        (q * 512, min(512, bn - q * 512))
                        for q in range((bn + 511) // 512)
                    ]
                    xg_sb = xpool.tile([P, HC, 1024], bf, tag="xg")
                    nc.sync.dma_start(
                        out=xg_sb[:, :, :bn], in_=xgt_r[:, :, s : s + bn]
                    )
                    act_sb = apool.tile([P, IC, 1024], bf, tag="act")
                    for i in range(IC):
                        pg = psg.tile([P, 1024], f32, tag="pg")
                        pu = psu.tile([P, 1024], f32, tag="pu")
                        for hc in range(HC):
                            for q0, qn in sbs:
                                nc.tensor.matmul(
                                    out=pg[:, q0 : q0 + qn],
                                    lhsT=wgu_sb[:, hc, i * P : (i + 1) * P],
                                    rhs=xg_sb[:, hc, q0 : q0 + qn],
                                    start=(hc == 0),
                                    stop=(hc == HC - 1),
                                )
                        for hc in range(HC):
                            for q0, qn in sbs:
                                nc.tensor.matmul(
                                    out=pu[:, q0 : q0 + qn],
                                    lhsT=wgu_sb[:, hc, I + i * P : I + (i + 1) * P],
                                    rhs=xg_sb[:, hc, q0 : q0 + qn],
                                    start=(hc == 0),
                                    stop=(hc == HC - 1),
                                )
                        sg = spool.tile([P, 1024], f32, tag="sg")
                        nc.scalar.activation(out=sg[:, :bn], in_=pg[:, :bn], func=Silu)
                        nc.vector.tensor_tensor(
                            out=act_sb[:, i, :bn], in0=sg[:, :bn], in1=pu[:, :bn], op=mult
                        )
                    for ts in range(bn // P):
                        ct = cpool.tile([P, 1], f32, tag="ct")
                        nc.sync.dma_start(
                            out=ct[:], in_=cv[s + ts * P : s + (ts + 1) * P, :]
                        )
                        po = pso.tile([P, 1024], f32, tag="po")
                        for i in range(IC):
                            for nh in range(2):
                                nc.tensor.matmul(
                                    out=po[:, nh * 512 : (nh + 1) * 512],
                                    lhsT=act_sb[:, i, ts * P : (ts + 1) * P],
                                    rhs=wd_sb[:, i, nh * 512 : (nh + 1) * 512],
                                    start=(i == 0),
                                    stop=(i == IC - 1),
                                )
                        ob = opool.tile([P, H], f32, tag="ob")
                        nc.vector.tensor_tensor(
                            out=ob[:],
                            in0=po[:],
                            in1=ct[:].to_broadcast([P, H]),
                            op=mult,
                        )
                        nc.sync.dma_start(
                            out=g[s + ts * P : s + (ts + 1) * P, :],
                            in_=ob[:],
                        )
                    off += bn
    nc.compile()
    return nc


def kernel(hidden_states, gate_weight, correction_bias, w_gate, w_up, w_down):
    global last_results
    from concourse.bass_utils import run_bass_kernel_spmd

    hidden = np.ascontiguousarray(np.asarray(hidden_states, np.float32))
    w_gate = np.asarray(w_gate, np.float32)
    w_up = np.asarray(w_up, np.float32)
    w_down = np.asarray(w_down, np.float32)

    topk_idx, topk_w = _routing(hidden, gate_weight, correction_bias)

    # Per-expert token lists (ascending), via stable sort of the (token, k) pairs.
    flat_e = topk_idx.ravel()
    order = np.argsort(flat_e, kind="stable")
    tokens_sorted = (order // K).astype(np.int64)
    weights_sorted = topk_w.ravel()[order]
    counts = np.bincount(flat_e, minlength=E)
    starts = np.zeros(E + 1, np.int64)
    np.cumsum(counts, out=starts[1:])

    # Assign each core's experts to capacity-sorted slots: slot j holds the
    # j-th largest expert of each core, so slot capacity = max over cores of
    # that order statistic (much tighter than a single uniform capacity).
    core_counts = counts.reshape(NCORES, EPC)
    slot_order = np.argsort(-core_counts, axis=1, kind="stable")  # [core, slot] -> local expert
    sorted_counts = np.take_along_axis(core_counts, slot_order, axis=1)
    caps = ((sorted_counts.max(axis=0) + P - 1) // P) * P  # [EPC]
    caps = np.maximum(caps, P)
    slot_blocks = []
    for j in range(EPC):
        Cj = int(caps[j])
        bl = [1024] * (Cj // 1024)
        if Cj % 1024:
            bl.append(Cj % 1024)
        slot_blocks.append(tuple(bl))

    key = tuple(slot_blocks)
    if key not in _program_cache:
        _program_cache[key] = _build_program([list(b) for b in slot_blocks])
    nc = _program_cache[key]

    seg_off = np.zeros(EPC + 1, np.int64)
    np.cumsum(caps, out=seg_off[1:])
    NC = int(seg_off[-1])

    in_maps = []
    tok_lists = []
    for c in range(NCORES):
        perm = np.zeros(NC, np.int64)
        cw = np.zeros((NC, 1), np.float32)
        toks_c = []
        wgu_c = np.empty((EPC, H, I2), BF16)
        wd_c = np.empty((EPC, I, H), BF16)
        for j in range(EPC):
            e = c * EPC + int(slot_order[c, j])
            n = counts[e]
            s = int(seg_off[j])
            te = tokens_sorted[starts[e] : starts[e] + n]
            perm[s : s + n] = te
            cw[s : s + n, 0] = weights_sorted[starts[e] : starts[e] + n]
            toks_c.append(te)
            wgu_c[j, :, :I] = w_gate[e].T.astype(BF16)
            wgu_c[j, :, I:] = w_up[e].T.astype(BF16)
            wd_c[j] = w_down[e].T.astype(BF16)
        tok_lists.append(toks_c)
        xgt = np.ascontiguousarray(hidden[perm].T).astype(BF16)
        in_maps.append({"xgt": xgt, "wgu": wgu_c, "wd": wd_c, "cv": cw})

    last_results = run_bass_kernel_spmd(nc, in_maps, list(range(NCORES)))

    out = np.zeros((T, H), np.float32)
    for c in range(NCORES):
        gc = last_results.results[c]["g"]
        for j in range(EPC):
            e = c * EPC + int(slot_order[c, j])
            n = counts[e]
            s = int(seg_off[j])
            out[tok_lists[c][j]] += gc[s : s + n]
    return out


# revision 8
# speedup vs baseline: 1.0159x; 1.0159x over previous
"""MoE (MiMo-V2) kernel for 8x Trainium2 NeuronCores.

Strategy (expert-parallel, per the sharding hint):
  - Host: grouped-topk routing (exact replica of the reference gate, run in
    fp32 on jax-cpu), then tokens are gathered per expert into fixed-capacity
    segments. Each of the 8 cores owns 8 experts.
  - Device (Bass/Tile, one SPMD program): for each local expert, stream its
    gathered tokens through gate/up matmuls (bf16 operands, fp32 PSUM
    accumulate), silu*mul on ACT/DVE, down matmul back to token-major
    layout, scale rows by the combine weights, write gathered rows out.
  - Host: scatter-add the gathered per-expert rows into the [T, H] output.
"""

import numpy as np
import ml_dtypes

T, H, E, I, K, G, KG = 16384, 1024, 64, 768, 8, 8, 4
P = 128
NCORES = 8
EPC = E // NCORES  # experts per core
HC = H // P  # 8 contraction chunks for gate/up
IC = I // P  # 6 contraction chunks for down
I2 = 2 * I  # fused gate+up output width

BF16 = ml_dtypes.bfloat16

_program_cache = {}
last_results = None  # BassKernelResults of the most recent launch (for test.py)


def _routing(hidden, gate_w, bias):
    """Exact replica of reference._grouped_topk on jax-cpu (fp32)."""
    import jax
    import jax.numpy as jnp

    cpu = jax.devices("cpu")[0]
    with jax.default_device(cpu):
        hidden = jnp.asarray(np.asarray(hidden), jnp.float32)
        gate_w = jnp.asarray(np.asarray(gate_w), jnp.float32)
        bias = jnp.asarray(np.asarray(bias), jnp.float32)
        logits = hidden @ gate_w.T
        scores = jax.nn.sigmoid(logits)
        s_choice = scores + bias[None, :]
        t, e = scores.shape
        grouped = s_choice.reshape(t, G, e // G)
        top2, _ = jax.lax.top_k(grouped, 2)
        group_scores = top2.sum(-1)
        _, gidx = jax.lax.top_k(group_scores, KG)
        gmask = jnp.zeros((t, G), jnp.float32).at[jnp.arange(t)[:, None], gidx].set(1.0)
        emask = jnp.repeat(gmask, e // G, axis=1)
        masked = jnp.where(emask > 0, s_choice, -jnp.inf)
        _, topk_idx = jax.lax.top_k(masked, K)
        topk_w = jnp.take_along_axis(scores, topk_idx, axis=1)
        topk_w = topk_w / (topk_w.sum(-1, keepdims=True) + 1e-20)
        return np.asarray(topk_idx), np.asarray(topk_w, np.float32)


def _build_program(slot_blocks):
    """One SPMD Bass program. slot_blocks[j] is the token-block decomposition
    of local-expert slot j; slots have (generally different) fixed capacities
    shared by all cores."""
    import concourse.mybir as mybir
    from concourse import bacc
    from concourse.tile import TileContext

    caps = [sum(b) for b in slot_blocks]
    seg_off = np.zeros(EPC + 1, np.int64)
    np.cumsum(caps, out=seg_off[1:])
    NC = int(seg_off[-1])
    bf = mybir.dt.bfloat16
    f32 = mybir.dt.float32
    Silu = mybir.ActivationFunctionType.Silu
    mult = mybir.AluOpType.mult

    nc = bacc.Bacc("TRN2", target_bir_lowering=False, debug=False, num_devices=NCORES)
    xgt = nc.dram_tensor("xgt", [H, NC], bf, kind="ExternalInput").ap()
    wgu = nc.dram_tensor("wgu", [EPC, H, I2], bf, kind="ExternalInput").ap()
    wd = nc.dram_tensor("wd", [EPC, I, H], bf, kind="ExternalInput").ap()
    cv = nc.dram_tensor("cv", [NC, 1], f32, kind="ExternalInput").ap()
    g = nc.dram_tensor("g", [NC, H], f32, kind="ExternalOutput").ap()

    with TileContext(nc) as tc:
        with (
            tc.tile_pool(name="wpool", bufs=2) as wpool,
            tc.tile_pool(name="xpool", bufs=2) as xpool,
            tc.tile_pool(name="apool", bufs=2) as apool,
            tc.tile_pool(name="spool", bufs=2) as spool,
            tc.tile_pool(name="opool", bufs=4) as opool,
            tc.tile_pool(name="cpool", bufs=4) as cpool,
            tc.tile_pool(name="psg", bufs=1, space="PSUM") as psg,
            tc.tile_pool(name="psu", bufs=1, space="PSUM") as psu,
            tc.tile_pool(name="pso", bufs=2, space="PSUM") as pso,
        ):
            xgt_r = xgt.rearrange("(c p) t -> p c t", p=P)  # [128, HC, NC]
            for ei in range(EPC):
                wgu_sb = wpool.tile([P, HC, I2], bf, tag="wgu")
                nc.sync.dma_start(
                    out=wgu_sb[:], in_=wgu[ei].rearrange("(c p) i -> p c i", p=P)
                )
                wd_sb = wpool.tile([P, IC, H], bf, tag="wd")
                nc.sync.dma_start(
                    out=wd_sb[:], in_=wd[ei].rearrange("(c p) h -> p c h", p=P)
                )
                off = 0
                for bn in slot_blocks[ei]:
                    s = int(seg_off[ei]) + off
                    # token sub-blocks of <=512 within this block; consecutive
                    # matmuls share one stationary (LDWEIGHTS) load across them
                    sbs = [
                        (q * 512, min(512, bn - q * 512))
                        for q in range((bn + 511) // 512)
                    ]
                    xg_sb = xpool.tile([P, HC, 1024], bf, tag="xg")
                    nc.sync.dma_start(
                        out=xg_sb[:, :, :bn], in_=xgt_r[:, :, s : s + bn]
                    )
                    act_sb = apool.tile([P, IC, 1024], bf, tag="act")
                    for i in range(IC):
                        pg = psg.tile([P, 1024], f32, tag="pg")
                        pu = psu.tile([P, 1024], f32, tag="pu")
                        for hc in range(HC):
                            for q0, qn in sbs:
                                nc.tensor.matmul(
                                    out=pg[:, q0 : q0 + qn],
                                    lhsT=wgu_sb[:, hc, i * P : (i + 1) * P],
                                    rhs=xg_sb[:, hc, q0 : q0 + qn],
                                    start=(hc == 0),
                                    stop=(hc == HC - 1),
                                )
                        for hc in range(HC):
                            for q0, qn in sbs:
                                nc.tensor.matmul(
                                    out=pu[:, q0 : q0 + qn],
                                    lhsT=wgu_sb[:, hc, I + i * P : I + (i + 1) * P],
                                    rhs=xg_sb[:, hc, q0 : q0 + qn],
                                    start=(hc == 0),
                                    stop=(hc == HC - 1),
                                )
                        sg = spool.tile([P, 1024], f32, tag="sg")
                        nc.scalar.activation(out=sg[:, :bn], in_=pg[:, :bn], func=Silu)
                        nc.vector.tensor_tensor(
                            out=act_sb[:, i, :bn], in0=sg[:, :bn], in1=pu[:, :bn], op=mult
                        )
                    for ts in range(bn // P):
                        ct = cpool.tile([P, 1], f32, tag="ct")
                        nc.sync.dma_start(
                            out=ct[:], in_=cv[s + ts * P : s + (ts + 1) * P, :]
                        )
                        po = pso.tile([P, 1024], f32, tag="po")
                        for i in range(IC):
                            for nh in range(2):
                                nc.tensor.matmul(
                                    out=po[:, nh * 512 : (nh + 1) * 512],
                                    lhsT=act_sb[:, i, ts * P : (ts + 1) * P],
                                    rhs=wd_sb[:, i, nh * 512 : (nh + 1) * 512],
                                    start=(i == 0),
                                    stop=(i == IC - 1),
                                )
                        ob = opool.tile([P, H], f32, tag="ob")
                        nc.vector.tensor_tensor(
                            out=ob[:],
                            in0=po[:],
                            in1=ct[:].to_broadcast([P, H]),
                            op=mult,
                        )
                        nc.sync.dma_start(
                            out=g[s + ts * P : s + (ts + 1) * P, :],
                            in_=ob[:],
                        )
                    off += bn
    nc.compile()
    return nc


def kernel(hidden_states, gate_weight, correction_bias, w_gate, w_up, w_down):
    global last_results
    from concourse.bass_utils import run_bass_kernel_spmd

    hidden = np.ascontiguousarray(np.asarray(hidden_states, np.float32))
    w_gate = np.asarray(w_gate, np.float32)
    w_up = np.asarray(w_up, np.float32)
    w_down = np.asarray(w_down, np.float32)

    topk_idx, topk_w = _routing(hidden, gate_weight, correction_bias)

    # Per-expert token lists (ascending), via stable sort of the (token, k) pairs.
    flat_e = topk_idx.ravel()
    order = np.argsort(flat_e, kind="stable")
    tokens_sorted = (order // K).astype(np.int64)
    weights_sorted = topk_w.ravel()[order]
    counts = np.bincount(flat_e, minlength=E)
    starts = np.zeros(E + 1, np.int64)
    np.cumsum(counts, out=starts[1:])

    # Assign each core's experts to capacity-sorted slots: slot j holds the
    # j-th largest expert of each core, so slot capacity = max over cores of
    # that order statistic (much tighter than a single uniform capacity).
    core_counts = counts.reshape(NCORES, EPC)
    slot_order = np.argsort(-core_counts, axis=1, kind="stable")  # [core, slot] -> local expert
    sorted_counts = np.take_along_axis(core_counts, slot_order, axis=1)
    caps = ((sorted_counts.max(axis=0) + P - 1) // P) * P  # [EPC]
    caps = np.maximum(caps, P)
    slot_blocks = []
    for j in range(EPC):
        Cj = int(caps[j])
        bl = [1024] * (Cj // 1024)
        if Cj % 1024:
            bl.append(Cj % 1024)
        slot_blocks.append(tuple(bl))

    print(f"[kernel] expert counts min/mean/max: {counts.min()}/{counts.mean():.0f}/{counts.max()}; "
          f"slot caps {list(map(int, caps))} sum {int(caps.sum())}")
    key = tuple(slot_blocks)
    if key not in _program_cache:
        _program_cache[key] = _build_program([list(b) for b in slot_blocks])
    nc = _program_cache[key]

    seg_off = np.zeros(EPC + 1, np.int64)
    np.cumsum(caps, out=seg_off[1:])
    NC = int(seg_off[-1])

    in_maps = []
    tok_lists = []
    for c in range(NCORES):
        perm = np.zeros(NC, np.int64)
        cw = np.zeros((NC, 1), np.float32)
        toks_c = []
        wgu_c = np.empty((EPC, H, I2), BF16)
        wd_c = np.empty((EPC, I, H), BF16)
        for j in range(EPC):
            e = c * EPC + int(slot_order[c, j])
            n = counts[e]
            s = int(seg_off[j])
            te = tokens_sorted[starts[e] : starts[e] + n]
            perm[s : s + n] = te
            cw[s : s + n, 0] = weights_sorted[starts[e] : starts[e] + n]
            toks_c.append(te)
            wgu_c[j, :, :I] = w_gate[e].T.astype(BF16)
            wgu_c[j, :, I:] = w_up[e].T.astype(BF16)
            wd_c[j] = w_down[e].T.astype(BF16)
        tok_lists.append(toks_c)
        xgt = np.ascontiguousarray(hidden[perm].T).astype(BF16)
        in_maps.append({"xgt": xgt, "wgu": wgu_c, "wd": wd_c, "cv": cw})

    last_results = run_bass_kernel_spmd(nc, in_maps, list(range(NCORES)))

    out = np.zeros((T, H), np.float32)
    for c in range(NCORES):
        gc = last_results.results[c]["g"]
        for j in range(EPC):
            e = c * EPC + int(slot_order[c, j])
            n = counts[e]
            s = int(seg_off[j])
            out[tok_lists[c][j]] += gc[s : s + n]
    return out


# revision 16
# speedup vs baseline: 1.1883x; 1.1697x over previous
"""MoE (MiMo-V2) kernel for 8x Trainium2 NeuronCores.

Strategy (expert-parallel, per the sharding hint):
  - Host: grouped-topk routing (exact replica of the reference gate, run in
    fp32 on jax-cpu), then tokens are gathered per expert into fixed-capacity
    segments. Each of the 8 cores owns 8 experts.
  - Device (Bass/Tile, one SPMD program): for each local expert, stream its
    gathered tokens through gate/up matmuls (bf16 operands, fp32 PSUM
    accumulate), silu*mul on ACT/DVE, down matmul back to token-major
    layout, scale rows by the combine weights, write gathered rows out.
  - Host: scatter-add the gathered per-expert rows into the [T, H] output.
"""

import numpy as np
import ml_dtypes

T, H, E, I, K, G, KG = 16384, 1024, 64, 768, 8, 8, 4
P = 128
NCORES = 8
EPC = E // NCORES  # experts per core
HC = H // P  # 8 contraction chunks for gate/up
IC = I // P  # 6 contraction chunks for down
I2 = 2 * I  # fused gate+up output width

BF16 = ml_dtypes.bfloat16

_program_cache = {}
last_results = None  # BassKernelResults of the most recent launch (for test.py)


def _routing(hidden, gate_w, bias):
    """Exact replica of reference._grouped_topk on jax-cpu (fp32)."""
    import jax
    import jax.numpy as jnp

    cpu = jax.devices("cpu")[0]
    with jax.default_device(cpu):
        hidden = jnp.asarray(np.asarray(hidden), jnp.float32)
        gate_w = jnp.asarray(np.asarray(gate_w), jnp.float32)
        bias = jnp.asarray(np.asarray(bias), jnp.float32)
        logits = hidden @ gate_w.T
        scores = jax.nn.sigmoid(logits)
        s_choice = scores + bias[None, :]
        t, e = scores.shape
        grouped = s_choice.reshape(t, G, e // G)
        top2, _ = jax.lax.top_k(grouped, 2)
        group_scores = top2.sum(-1)
        _, gidx = jax.lax.top_k(group_scores, KG)
        gmask = jnp.zeros((t, G), jnp.float32).at[jnp.arange(t)[:, None], gidx].set(1.0)
        emask = jnp.repeat(gmask, e // G, axis=1)
        masked = jnp.where(emask > 0, s_choice, -jnp.inf)
        _, topk_idx = jax.lax.top_k(masked, K)
        topk_w = jnp.take_along_axis(scores, topk_idx, axis=1)
        topk_w = topk_w / (topk_w.sum(-1, keepdims=True) + 1e-20)
        return np.asarray(topk_idx), np.asarray(topk_w, np.float32)


def _build_program(slot_blocks):
    """One SPMD Bass program. slot_blocks[j] is the token-block decomposition
    of local-expert slot j; slots have (generally different) fixed capacities
    shared by all cores."""
    import concourse.mybir as mybir
    from concourse import bacc
    from concourse.tile import TileContext

    caps = [sum(b) for b in slot_blocks]
    seg_off = np.zeros(EPC + 1, np.int64)
    np.cumsum(caps, out=seg_off[1:])
    NC = int(seg_off[-1])
    bf = mybir.dt.bfloat16
    f32 = mybir.dt.float32
    Silu = mybir.ActivationFunctionType.Silu
    mult = mybir.AluOpType.mult

    nc = bacc.Bacc("TRN2", target_bir_lowering=False, debug=False, num_devices=NCORES)
    xgt = nc.dram_tensor("xgt", [H, NC], bf, kind="ExternalInput").ap()
    wgu = nc.dram_tensor("wgu", [EPC, H, I2], bf, kind="ExternalInput").ap()
    wd = nc.dram_tensor("wd", [EPC, I, H], bf, kind="ExternalInput").ap()
    cv = nc.dram_tensor("cv", [NC, 1], f32, kind="ExternalInput").ap()
    g = nc.dram_tensor("g", [NC, H], f32, kind="ExternalOutput").ap()

    with TileContext(nc) as tc:
        with (
            tc.tile_pool(name="wpool", bufs=2) as wpool,
            tc.tile_pool(name="xpool", bufs=2) as xpool,
            tc.tile_pool(name="apool", bufs=2) as apool,
            tc.tile_pool(name="spool", bufs=2) as spool,
            tc.tile_pool(name="opool", bufs=4) as opool,
            tc.tile_pool(name="cpool", bufs=4) as cpool,
            tc.tile_pool(name="psg", bufs=1, space="PSUM") as psg,
            tc.tile_pool(name="psu", bufs=1, space="PSUM") as psu,
            tc.tile_pool(name="pso", bufs=2, space="PSUM") as pso,
        ):
            xgt_r = xgt.rearrange("(c p) t -> p c t", p=P)  # [128, HC, NC]
            for ei in range(EPC):
                wgu_r = wgu[ei].rearrange("(c p) i -> c p i", p=P)
                wgu_sb = []
                for hc in range(HC):
                    w = wpool.tile([P, I2], bf, tag=f"wgu{hc}")
                    nc.sync.dma_start(out=w[:], in_=wgu_r[hc])
                    wgu_sb.append(w)
                wd_r = wd[ei].rearrange("(c p) h -> c p h", p=P)
                wd_sb = []
                for ic in range(IC):
                    w = wpool.tile([P, H], bf, tag=f"wd{ic}")
                    nc.sync.dma_start(out=w[:], in_=wd_r[ic])
                    wd_sb.append(w)
                off = 0
                for bn in slot_blocks[ei]:
                    s = int(seg_off[ei]) + off
                    # token sub-blocks of <=512 within this block; consecutive
                    # matmuls share one stationary (LDWEIGHTS) load across them
                    sbs = [
                        (q * 512, min(512, bn - q * 512))
                        for q in range((bn + 511) // 512)
                    ]
                    xg_sb = []
                    for hc in range(HC):
                        xt = xpool.tile([P, 1024], bf, tag=f"xg{hc}")
                        nc.sync.dma_start(
                            out=xt[:, :bn], in_=xgt_r[:, hc, s : s + bn]
                        )
                        xg_sb.append(xt)
                    act_sb = apool.tile([P, IC, 1024], bf, tag="act")
                    for i in range(IC):
                        pg = psg.tile([P, 1024], f32, tag="pg")
                        pu = psu.tile([P, 1024], f32, tag="pu")
                        for hc in range(HC):
                            for q0, qn in sbs:
                                nc.tensor.matmul(
                                    out=pg[:, q0 : q0 + qn],
                                    lhsT=wgu_sb[hc][:, i * P : (i + 1) * P],
                                    rhs=xg_sb[hc][:, q0 : q0 + qn],
                                    start=(hc == 0),
                                    stop=(hc == HC - 1),
                                )
                        for hc in range(HC):
                            for q0, qn in sbs:
                                nc.tensor.matmul(
                                    out=pu[:, q0 : q0 + qn],
                                    lhsT=wgu_sb[hc][:, I + i * P : I + (i + 1) * P],
                                    rhs=xg_sb[hc][:, q0 : q0 + qn],
                                    start=(hc == 0),
                                    stop=(hc == HC - 1),
                                )
                        sg = spool.tile([P, 1024], f32, tag="sg")
                        nc.scalar.activation(out=sg[:, :bn], in_=pg[:, :bn], func=Silu)
                        nc.vector.tensor_tensor(
                            out=act_sb[:, i, :bn], in0=sg[:, :bn], in1=pu[:, :bn], op=mult
                        )
                    for ts in range(bn // P):
                        ct = cpool.tile([P, 1], f32, tag="ct")
                        nc.sync.dma_start(
                            out=ct[:], in_=cv[s + ts * P : s + (ts + 1) * P, :]
                        )
                        po = pso.tile([P, 1024], f32, tag="po")
                        for i in range(IC):
                            for nh in range(2):
                                nc.tensor.matmul(
                                    out=po[:, nh * 512 : (nh + 1) * 512],
                                    lhsT=act_sb[:, i, ts * P : (ts + 1) * P],
                                    rhs=wd_sb[i][:, nh * 512 : (nh + 1) * 512],
                                    start=(i == 0),
                                    stop=(i == IC - 1),
                                )
                        ob = opool.tile([P, H], f32, tag="ob")
                        nc.vector.tensor_tensor(
                            out=ob[:],
                            in0=po[:],
                            in1=ct[:].to_broadcast([P, H]),
                            op=mult,
                        )
                        nc.sync.dma_start(
                            out=g[s + ts * P : s + (ts + 1) * P, :],
                            in_=ob[:],
                        )
                    off += bn
    nc.compile()
    return nc


def kernel(hidden_states, gate_weight, correction_bias, w_gate, w_up, w_down):
    global last_results
    from concourse.bass_utils import run_bass_kernel_spmd

    hidden = np.ascontiguousarray(np.asarray(hidden_states, np.float32))
    w_gate = np.asarray(w_gate, np.float32)
    w_up = np.asarray(w_up, np.float32)
    w_down = np.asarray(w_down, np.float32)

    topk_idx, topk_w = _routing(hidden, gate_weight, correction_bias)

    # Per-expert token lists (ascending), via stable sort of the (token, k) pairs.
    flat_e = topk_idx.ravel()
    order = np.argsort(flat_e, kind="stable")
    tokens_sorted = (order // K).astype(np.int64)
    weights_sorted = topk_w.ravel()[order]
    counts = np.bincount(flat_e, minlength=E)
    starts = np.zeros(E + 1, np.int64)
    np.cumsum(counts, out=starts[1:])

    # Snake-assign experts to cores by descending token count (balances the
    # per-core load), then give each core's j-th largest expert slot j. Slot
    # capacity = max over cores of that order statistic, which with the snake
    # assignment is close to the global (8j)-th order statistic — near-minimal
    # uniform-program padding.
    rank = np.argsort(-counts, kind="stable")
    core_experts = [[] for _ in range(NCORES)]
    for r, e in enumerate(rank):
        blk, pos = divmod(r, NCORES)
        c = pos if blk % 2 == 0 else NCORES - 1 - pos
        core_experts[c].append(int(e))
    # slot j of core c = j-th largest expert of that core (snake emits them
    # in descending order already)
    slot_expert = np.array(core_experts)  # [NCORES, EPC], desc count order
    sorted_counts = counts[slot_expert]
    caps = ((sorted_counts.max(axis=0) + P - 1) // P) * P  # [EPC]
    caps = np.maximum(caps, P)
    slot_blocks = []
    for j in range(EPC):
        Cj = int(caps[j])
        bl = [1024] * (Cj // 1024)
        if Cj % 1024:
            bl.append(Cj % 1024)
        slot_blocks.append(tuple(bl))

    print(f"[kernel] expert counts min/mean/max: {counts.min()}/{counts.mean():.0f}/{counts.max()}; "
          f"slot caps {list(map(int, caps))} sum {int(caps.sum())}")
    key = tuple(slot_blocks)
    if key not in _program_cache:
        _program_cache[key] = _build_program([list(b) for b in slot_blocks])
    nc = _program_cache[key]

    seg_off = np.zeros(EPC + 1, np.int64)
    np.cumsum(caps, out=seg_off[1:])
    NC = int(seg_off[-1])

    in_maps = []
    tok_lists = []
    for c in range(NCORES):
        perm = np.zeros(NC, np.int64)
        cw = np.zeros((NC, 1), np.float32)
        toks_c = []
        wgu_c = np.empty((EPC, H, I2), BF16)
        wd_c = np.empty((EPC, I, H), BF16)
        for j in range(EPC):
            e = int(slot_expert[c, j])
            n = counts[e]
            s = int(seg_off[j])
            te = tokens_sorted[starts[e] : starts[e] + n]
            perm[s : s + n] = te
            cw[s : s + n, 0] = weights_sorted[starts[e] : starts[e] + n]
            toks_c.append(te)
            wgu_c[j, :, :I] = w_gate[e].T.astype(BF16)
            wgu_c[j, :, I:] = w_up[e].T.astype(BF16)
            wd_c[j] = w_down[e].T.astype(BF16)
        tok_lists.append(toks_c)
        xgt = np.ascontiguousarray(hidden[perm].T).astype(BF16)
        in_maps.append({"xgt": xgt, "wgu": wgu_c, "wd": wd_c, "cv": cw})

    last_results = run_bass_kernel_spmd(nc, in_maps, list(range(NCORES)))

    out = np.zeros((T, H), np.float32)
    for c in range(NCORES):
        gc = last_results.results[c]["g"]
        for j in range(EPC):
            e = int(slot_expert[c, j])
            n = counts[e]
            s = int(seg_off[j])
            out[tok_lists[c][j]] += gc[s : s + n]
    return out
